# revision 1
# baseline (speedup 1.0000x reference)
"""Trainium2 Bass kernel for nn_CircuitLayer (GNN message passing / KCL circuit).

res[b, n] = sum over edges e: (+i_e at des, -i_e at src),
i_e = a_e * tanh(w_e * (v_src - v_des) + b_e),  v = [0, x][node]

Strategy (node-parallel over 8 NeuronCores):
  - Node slots [0, 50176) split: NC i owns 6272 slots (8 Q7 cores x 784 nodes,
    28 tiles of 28 nodes each).
  - Every edge-endpoint incidence is routed to the (NC, core, tile) owning its
    node, sorted/grouped by node; sign folding: src-incidence w'=+w, a'=-a;
    des-incidence w'=-w, a'=+a; contribution c = a'*tanh(w'*(v_own-v_other)+b).
  - Per tile the device: ap_gathers v_other/v_own from a per-core compact table
    (distinct endpoints, int16-indexable), computes c on DVE/ACT (bf16),
    prefix-scans c (f32 state) and gathers per-node segment boundary sums.
  - Per-NC outputs are disjoint node ranges -> no collective needed.
"""

import numpy as np

B, N, E = 16, 50000, 1600000
NN = N + 1
NCS = 8
QC = 8
NPT = 28
TPC = 28
ROUNDS = 4
TPR = TPC // ROUNDS
NPC = NPT * TPC          # 784 nodes per core
NPNC = NPC * QC          # 6272 node slots per NC
MAX_CLEN = 32768 - 16

_cache = {}


def _pad16(n):
    return (n + 15) & ~15


def _bf16(x):
    x = np.ascontiguousarray(x, np.float32)
    u = x.view(np.uint32)
    r = ((u >> 16) & 1) + 0x7FFF
    return ((u + r) & 0xFFFF0000).view(np.float32)


def _wrap16(v):
    # (S,) -> (16, S//16): out[p, s] = v[s*16 + p]
    return v.reshape(-1, 16).T.copy()


def _preprocess(x, param, src_node, des_node):
    import ml_dtypes

    src = np.asarray(src_node).astype(np.int64)
    des = np.asarray(des_node).astype(np.int64)
    a, w, b = (np.asarray(param[i], np.float32) for i in range(3))

    own = np.concatenate([src, des])
    other = np.concatenate([des, src])
    wp = np.concatenate([w, -w])
    ap_ = np.concatenate([-a, a])
    bp = np.concatenate([b, b])

    order = np.argsort(own, kind="stable")
    own, other = own[order], other[order]
    wp, ap_, bp = wp[order], ap_[order], bp[order]

    cnt = np.bincount(own, minlength=NN).astype(np.int64)
    cstart = np.zeros(NN + 1, np.int64)
    np.cumsum(cnt, out=cstart[1:])

    # global tile capacity
    tile_tot = np.bincount(np.arange(NN) // NPT, weights=cnt,
                           minlength=(NCS * QC * TPC))
    CAP = _pad16(int(tile_tot.max()) + 1 + 16)
    assert CAP <= 4096, CAP

    aux = np.concatenate([np.zeros((B, 1), np.float32),
                          np.asarray(x, np.float32)], axis=1)

    # ---- per (nc, core, round): distinct endpoint lists ----
    dls = [[[None] * QC for _ in range(ROUNDS)] for _ in range(NCS)]
    clen_need = 0
    for nc in range(NCS):
        for r in range(ROUNDS):
            for k in range(QC):
                n0 = nc * NPNC + k * NPC + r * TPR * NPT
                n1 = min(n0 + TPR * NPT, NN)
                if n0 >= NN:
                    dls[nc][r][k] = np.empty(0, np.int64)
                    continue
                s, e = cstart[n0], cstart[n1]
                u = np.unique(np.concatenate([other[s:e], own[s:e]]))
                dls[nc][r][k] = u
                clen_need = max(clen_need, len(u))
    CLEN = _pad16(clen_need)
    assert CLEN <= MAX_CLEN, CLEN

    IDXW = CAP // 16
    per_nc = []
    for nc in range(NCS):
        ctab = np.zeros((ROUNDS, 128, CLEN), np.float32)
        idxs = np.zeros((TPC, 128, 2 * IDXW + 2), np.int16)
        prm = np.zeros((TPC, 128, 5 * CAP), np.float32)
        for r in range(ROUNDS):
            for k in range(QC):
                dl = dls[nc][r][k]
                if len(dl):
                    ctab[r, 16 * k:16 * k + 16, :len(dl)] = aux[:, dl]
                for ti in range(TPR):
                    t = r * TPR + ti
                    n0 = nc * NPNC + k * NPC + t * NPT
                    ob = np.zeros(CAP, np.int16)
                    nb = np.zeros(CAP, np.int16)
                    wrow = np.zeros(CAP, np.float32)
                    brow = np.zeros(CAP, np.float32)
                    arow = np.zeros(CAP, np.float32)
                    mrow = np.ones(CAP, np.float32)
                    mrow[0] = 0.0
                    vrow = np.zeros((16, CAP), np.float32)
                    cnts = np.zeros(NPT, np.int64)
                    if n0 < NN:
                        n1 = min(n0 + NPT, NN)
                        s, e = cstart[n0], cstart[n1]
                        m = e - s
                        assert m + 1 <= CAP
                        ob[1:1 + m] = np.searchsorted(dl, other[s:e])
                        nb[1:1 + m] = np.searchsorted(dl, own[s:e])
                        wrow[1:1 + m] = wp[s:e]
                        brow[1:1 + m] = bp[s:e]
                        arow[1:1 + m] = ap_[s:e]
                        cnts[:n1 - n0] = cnt[n0:n1]
                        if m:
                            o_sl = own[s:e]
                            starts = np.ones(m, bool)
                            starts[1:] = o_sl[1:] != o_sl[:-1]
                            spos = np.nonzero(starts)[0] + 1
                            mrow[spos] = 0.0
                            vrow[:, spos] = aux[:, o_sl[starts]]
                    ends = np.zeros(32, np.int16)
                    ends[:NPT] = np.cumsum(cnts).astype(np.int16)
                    sl = slice(16 * k, 16 * k + 16)
                    idxs[t, sl, 0:IDXW] = _wrap16(ob)
                    idxs[t, sl, IDXW:2 * IDXW] = _wrap16(nb)
                    idxs[t, sl, 2 * IDXW:] = _wrap16(ends)
                    prm[t, sl, 0:CAP] = wrow
                    prm[t, sl, CAP:2 * CAP] = brow
                    prm[t, sl, 2 * CAP:3 * CAP] = arow
                    prm[t, sl, 3 * CAP:4 * CAP] = mrow
                    prm[t, sl, 4 * CAP:5 * CAP] = vrow
        per_nc.append(dict(
            ctab=ctab,
            idxs=idxs,
            prm=_bf16(prm).astype(ml_dtypes.bfloat16),
        ))
    return dict(CAP=CAP, CLEN=CLEN), per_nc


def _build_program(CAP, CLEN, repeat=1):
    import sys
    if "/opt/trn_rl_repo" not in sys.path:
        sys.path.insert(0, "/opt/trn_rl_repo")
    from contextlib import ExitStack
    from concourse import bass, bacc, mybir, tile

    f32 = mybir.dt.float32
    bf16 = mybir.dt.bfloat16
    i16 = mybir.dt.int16
    Alu = mybir.AluOpType
    IDXW = CAP // 16

    nc = bacc.Bacc("TRN2", target_bir_lowering=False, debug=False,
                   num_devices=NCS)
    ctab_d = nc.dram_tensor("ctab_in", [ROUNDS, 128, CLEN], f32,
                            kind="ExternalInput")
    idxs_d = nc.dram_tensor("idxs_in", [TPC, 128, 2 * IDXW + 2], i16,
                            kind="ExternalInput")
    prm_d = nc.dram_tensor("prm_in", [TPC, 128, 5 * CAP], bf16,
                           kind="ExternalInput")
    out_d = nc.dram_tensor("res_out", [128, TPC * NPT], f32,
                           kind="ExternalOutput")

    with tile.TileContext(nc) as tc, ExitStack() as ctx:
        ctab_p = ctx.enter_context(tc.tile_pool(name="ctab", bufs=1))
        gat_p = ctx.enter_context(tc.tile_pool(name="gat", bufs=2))
        in_p = ctx.enter_context(tc.tile_pool(name="inp", bufs=2))
        zz_p = ctx.enter_context(tc.tile_pool(name="zz", bufs=2))
        p_p = ctx.enter_context(tc.tile_pool(name="pp", bufs=2))
        e_p = ctx.enter_context(tc.tile_pool(name="ee", bufs=2))
        res_p = ctx.enter_context(tc.tile_pool(name="res", bufs=1))

        res = res_p.tile([128, TPC * NPT], f32, tag="res")
        for _rep in range(repeat):
         for r in range(ROUNDS):
            ctab = ctab_p.tile([128, CLEN], f32, tag="ctab")
            nc.sync.dma_start(ctab[:], ctab_d.ap()[r])
            for ti in range(TPR):
                t = r * TPR + ti
                idx = in_p.tile([128, 2 * IDXW + 2], i16, tag="idx")
                nc.sync.dma_start(idx[:], idxs_d.ap()[t])
                prm = in_p.tile([128, 5 * CAP], bf16, tag="prm")
                nc.sync.dma_start(prm[:], prm_d.ap()[t])

                go = gat_p.tile([128, CAP], f32, tag="go")
                nc.gpsimd.ap_gather(go[:], ctab[:], idx[:, 0:IDXW],
                                    128, CLEN, 1, CAP)
                gn = gat_p.tile([128, CAP], f32, tag="gn")
                nc.vector.tensor_tensor_scan(gn[:], prm[:, 3 * CAP:4 * CAP],
                                             prm[:, 4 * CAP:5 * CAP], 0.0,
                                             Alu.mult, Alu.add)

                z1 = zz_p.tile([128, CAP], bf16, tag="zz")
                nc.vector.tensor_tensor(z1[:], gn[:], go[:], Alu.subtract)
                z2 = zz_p.tile([128, CAP], bf16, tag="zz")
                nc.vector.tensor_tensor(z2[:], z1[:], prm[:, 0:CAP], Alu.mult)
                z3 = zz_p.tile([128, CAP], bf16, tag="zz")
                nc.vector.tensor_tensor(z3[:], z2[:], prm[:, CAP:2 * CAP],
                                        Alu.add)
                th = zz_p.tile([128, CAP], bf16, tag="zz")
                nc.scalar.activation(th[:], z3[:],
                                     mybir.ActivationFunctionType.Tanh)
                cc = zz_p.tile([128, CAP], bf16, tag="zz")
                nc.vector.tensor_tensor(cc[:], th[:], prm[:, 2 * CAP:3 * CAP],
                                        Alu.mult)
                P = p_p.tile([128, CAP], f32, tag="P")
                nc.vector.tensor_tensor_scan(P[:], cc[:], cc[:], 0.0,
                                             Alu.add, Alu.bypass)
                Eb = e_p.tile([128, 48], f32, tag="Eb")
                nc.vector.memset(Eb[:, 0:1], 0.0)
                nc.gpsimd.ap_gather(Eb[:, 1:33], P[:],
                                    idx[:, 2 * IDXW:2 * IDXW + 2],
                                    128, CAP, 1, 32)
                nc.vector.tensor_tensor(res[:, t * NPT:(t + 1) * NPT],
                                        Eb[:, 1:1 + NPT], Eb[:, 0:NPT],
                                        Alu.subtract)
        nc.sync.dma_start(out_d.ap()[:], res[:])
    nc.compile()
    return nc


def kernel(**inputs) -> np.ndarray:
    import sys
    if "/opt/trn_rl_repo" not in sys.path:
        sys.path.insert(0, "/opt/trn_rl_repo")
    from concourse.bass_utils import run_bass_kernel_spmd

    x = np.asarray(inputs["x"], np.float32)
    param = np.asarray(inputs["param"], np.float32)
    meta, per_nc = _preprocess(x, param, inputs["src_node"],
                               inputs["des_node"])
    key = (meta["CAP"], meta["CLEN"])
    if key not in _cache:
        _cache[key] = _build_program(*key)
    nc = _cache[key]

    in_maps = [{"ctab_in": d["ctab"], "idxs_in": d["idxs"],
                "prm_in": d["prm"]} for d in per_nc]
    results = run_bass_kernel_spmd(nc, in_maps, list(range(NCS))).results

    full = np.zeros((B, NCS * NPNC), np.float32)
    for i, om in enumerate(results):
        o = om["res_out"]
        for k in range(QC):
            full[:, i * NPNC + k * NPC:i * NPNC + (k + 1) * NPC] = \
                o[16 * k:16 * k + 16]
    return np.ascontiguousarray(full[:, 1:NN])



# revision 3
# speedup vs baseline: 13.2387x; 13.2387x over previous
"""Trainium2 Bass kernel v2 for nn_CircuitLayer (GNN message passing / KCL).

res[b, n] = sum over incident edges e of node n: sgn * a_e*tanh(w_e*dv + b_e)

Design (node-parallel over 8 NeuronCores, zero gpsimd work):
  - All NN=50001 node slots (padded to 50176) are sorted by incidence degree
    and dealt into 28 rounds x 8 NCs x (8 groups x 28 nodes) blocks. Within a
    round every NC uses the same window G(r) = max degree in the round, so one
    SPMD program serves all cores.
  - Host packs, per (NC, round): folded params w',b',a' (sign-folded per
    incidence endpoint) and gathered v_other, all fp16, into fixed windows of
    G slots per node (padding has a'=w'=b'=0 => contributes 0).
  - Device per tile: d = bcast(v_own) - v_other; z = d*w' + b'; th = tanh(z)
    on ACT; c = th*a'; windowed tensor_reduce(add) -> per-node sums. All on
    DVE/ACT; no gather, no scan, no gpsimd.
  - Per-NC outputs are disjoint node sets -> no collective; host inverse
    permutation assembles the full (16, 50000) result.
"""

import numpy as np

B, N, E = 16, 50000, 1600000
NN = N + 1
NCS = 8
QC = 8
NPT = 28
RND = 28
BLK = QC * NPT            # 224 nodes per (NC, round) block
SLOTS = NCS * RND * BLK   # 50176 node slots

_cache = {}


def _preprocess(x, param, src_node, des_node):
    src = np.asarray(src_node).astype(np.int64)
    des = np.asarray(des_node).astype(np.int64)
    a, w, b = (np.asarray(param[i], np.float32) for i in range(3))

    own = np.concatenate([src, des])
    other = np.concatenate([des, src])
    wp = np.concatenate([w, -w])
    ap_ = np.concatenate([-a, a])
    bp = np.concatenate([b, b])

    order = np.argsort(own, kind="stable")
    other_s = other[order].astype(np.int64)
    wp_s = wp[order].astype(np.float16)
    ap_s = ap_[order].astype(np.float16)
    bp_s = bp[order].astype(np.float16)

    deg = np.bincount(own, minlength=NN).astype(np.int64)
    cstart = np.zeros(NN + 1, np.int64)
    np.cumsum(deg, out=cstart[1:])

    deg_pad = np.concatenate([deg, np.full(SLOTS - NN, -1, np.int64)])
    nodesort = np.argsort(-deg_pad, kind="stable")
    degsrt = deg_pad[nodesort]

    G_list = []
    for r in range(RND):
        g = int(max(1, degsrt[r * 8 * BLK: (r + 1) * 8 * BLK].max()))
        g += g % 2
        G_list.append(g)
    CAPs = [NPT * g for g in G_list]
    offs = np.concatenate([[0], np.cumsum(CAPs)]).astype(np.int64)
    TOT = int(offs[-1])

    aux16 = np.concatenate(
        [np.zeros((B, 1), np.float32), np.asarray(x, np.float32)],
        axis=1).astype(np.float16)

    per_nc = []
    karr = np.arange(BLK) // NPT
    for i in range(NCS):
        big = np.zeros((128, 4 * TOT), np.float16)
        v1 = np.zeros((128, RND * NPT), np.float16)
        for r in range(RND):
            G = G_list[r]
            CAP = CAPs[r]
            o4 = 4 * offs[r]
            bi = (8 * r + i) * BLK
            nodes = nodesort[bi: bi + BLK]
            nid_safe = np.where(nodes < NN, nodes, 0)
            reps = np.clip(deg_pad[nodes], 0, None)
            tot = int(reps.sum())
            cum = np.cumsum(reps)
            within = np.arange(tot) - np.repeat(cum - reps, reps)
            srcpos = np.repeat(cstart[nid_safe], reps) + within
            j_of = np.repeat(np.arange(BLK), reps)
            k_of = karr[j_of]
            col = (j_of % NPT) * G + within

            f8 = np.zeros((3, QC, CAP), np.float16)
            f8[0, k_of, col] = wp_s[srcpos]
            f8[1, k_of, col] = bp_s[srcpos]
            f8[2, k_of, col] = ap_s[srcpos]
            oth = np.zeros((QC, CAP), np.int64)
            oth[k_of, col] = other_s[srcpos]

            big[:, o4 + 0 * CAP: o4 + 1 * CAP] = np.repeat(f8[0], 16, 0)
            big[:, o4 + 1 * CAP: o4 + 2 * CAP] = np.repeat(f8[1], 16, 0)
            big[:, o4 + 2 * CAP: o4 + 3 * CAP] = np.repeat(f8[2], 16, 0)
            for k in range(QC):
                big[16 * k: 16 * k + 16, o4 + 3 * CAP: o4 + 4 * CAP] = \
                    aux16[:, oth[k]]
                v1[16 * k: 16 * k + 16, r * NPT: (r + 1) * NPT] = \
                    aux16[:, nid_safe[k * NPT: (k + 1) * NPT]]
        per_nc.append(dict(big=big, v1=v1))
    return dict(G_list=tuple(G_list), TOT=TOT, nodesort=nodesort), per_nc


def _build_program(G_list, repeat=1):
    import sys
    if "/opt/trn_rl_repo" not in sys.path:
        sys.path.insert(0, "/opt/trn_rl_repo")
    from contextlib import ExitStack
    from concourse import bass, bacc, mybir, tile

    f16 = mybir.dt.float16
    f32 = mybir.dt.float32
    Alu = mybir.AluOpType
    CAPs = [NPT * g for g in G_list]
    TOT = sum(CAPs)

    nc = bacc.Bacc("TRN2", target_bir_lowering=False, debug=False,
                   num_devices=NCS)
    big_d = nc.dram_tensor("big_in", [128, 4 * TOT], f16,
                           kind="ExternalInput")
    v1_d = nc.dram_tensor("v1_in", [128, RND * NPT], f16,
                          kind="ExternalInput")
    out_d = nc.dram_tensor("res_out", [128, RND * NPT], f32,
                           kind="ExternalOutput")

    with tile.TileContext(nc) as tc, ExitStack() as ctx:
        big_p = ctx.enter_context(tc.tile_pool(name="big", bufs=3))
        dd_p = ctx.enter_context(tc.tile_pool(name="dd", bufs=2))
        zz_p = ctx.enter_context(tc.tile_pool(name="zz", bufs=2))
        res_p = ctx.enter_context(tc.tile_pool(name="res", bufs=1))
        v1_p = ctx.enter_context(tc.tile_pool(name="v1", bufs=1))

        v1 = v1_p.tile([128, RND * NPT], f16, tag="v1")
        nc.sync.dma_start(v1[:], v1_d.ap())
        res = res_p.tile([128, RND * NPT], f32, tag="res")
        for _rep in range(repeat):
            off = 0
            for r, G in enumerate(G_list):
                CAP = CAPs[r]
                big = big_p.tile([128, 4 * CAP], f16, tag="big")
                nc.sync.dma_start(big[:], big_d.ap()[:, 4 * off:
                                                     4 * off + 4 * CAP])
                d = dd_p.tile([128, CAP], f16, tag="dd")
                v1bc = v1[:, r * NPT:(r + 1) * NPT].to_broadcast(
                    [128, NPT, G])
                nc.vector.tensor_tensor(
                    d[:].rearrange("p (a g) -> p a g", g=G), v1bc,
                    big[:, 3 * CAP:4 * CAP].rearrange("p (a g) -> p a g",
                                                      g=G),
                    Alu.subtract)
                z1 = zz_p.tile([128, CAP], f16, tag="zz")
                nc.vector.tensor_tensor(z1[:], d[:], big[:, 0:CAP], Alu.mult)
                z2 = zz_p.tile([128, CAP], f16, tag="zz")
                nc.vector.tensor_tensor(z2[:], z1[:], big[:, CAP:2 * CAP],
                                        Alu.add)
                th = zz_p.tile([128, CAP], f16, tag="zz")
                nc.scalar.activation(th[:], z2[:],
                                     mybir.ActivationFunctionType.Tanh)
                cc = zz_p.tile([128, CAP], f16, tag="zz")
                nc.vector.tensor_tensor(cc[:], th[:], big[:, 2 * CAP:3 * CAP],
                                        Alu.mult)
                nc.vector.tensor_reduce(
                    res[:, r * NPT:(r + 1) * NPT],
                    cc[:].rearrange("p (a g) -> p a g", g=G),
                    axis=mybir.AxisListType.X, op=Alu.add)
                off += CAP
        nc.sync.dma_start(out_d.ap(), res[:])
    nc.compile()
    return nc


def _assemble(results, nodesort):
    full = np.zeros((B, NN), np.float32)
    r_g = np.arange(RND)[:, None, None]
    k_g = np.arange(QC)[None, :, None]
    j_g = np.arange(NPT)[None, None, :]
    for i in range(NCS):
        o = np.asarray(results[i]["res_out"]).reshape(QC, 16, RND, NPT)
        nodes = nodesort[(8 * r_g + i) * BLK + NPT * k_g + j_g]  # (RND,QC,NPT)
        vals = o.transpose(1, 2, 0, 3)                           # (B,RND,QC,NPT)
        m = nodes < NN
        full[:, nodes[m]] = vals[:, m]
    return np.ascontiguousarray(full[:, 1:])


def _in_map(d):
    return {"big_in": d["big"], "v1_in": d["v1"]}


def kernel(**inputs) -> np.ndarray:
    import sys
    if "/opt/trn_rl_repo" not in sys.path:
        sys.path.insert(0, "/opt/trn_rl_repo")
    from concourse.bass_utils import run_bass_kernel_spmd

    x = np.asarray(inputs["x"], np.float32)
    param = np.asarray(inputs["param"], np.float32)
    meta, per_nc = _preprocess(x, param, inputs["src_node"],
                               inputs["des_node"])
    key = meta["G_list"]
    if key not in _cache:
        _cache[key] = _build_program(key)
    nc = _cache[key]

    in_maps = [_in_map(d) for d in per_nc]
    results = run_bass_kernel_spmd(nc, in_maps, list(range(NCS))).results
    return _assemble(results, meta["nodesort"])
